# revision 1
# baseline (speedup 1.0000x reference)
"""DispEstimator Trainium2 kernel: 8-core SPMD (batch x H-half sharding).

Core c handles sample b=c//2, vertical half h=c%2. Odd cores get a vertically
flipped view of their sample (dy-flipped weights, permuted lc-channel order
for e1), so every core runs identical code: the "top" slab edge is a true
image boundary, the "bottom" edge is interior halo. Instance-norm and
correlation-normalization statistics are made global via pairwise AllReduce
between the two cores of each sample.

Correlation: lc_k = (A + B_shift - 2*C_k)/64 with A = sum_c f2^2,
B = sum_c f1s^2, C_k = sum_c f2 * shift_k(f1s). C is computed on
DVE (products) + TensorE (pairwise block-ones reduction). A rides as an extra
e1 input channel (summed weights); B's contribution is folded into a 9x9
single-channel conv on B executed as extra accumulating matmuls in e1. The
1/(64*s) normalization is baked into the e1/B weight tiles after AllReduce.
"""
import sys

import numpy as np

if "/opt/trn_rl_repo" not in sys.path:
    sys.path.insert(0, "/opt/trn_rl_repo")

import bass_rust
import concourse.bass as bass
import concourse.mybir as mybir
from concourse.bass_utils import run_bass_kernel_spmd
from concourse.tile import TileContext

F32 = mybir.dt.float32
F32R = mybir.dt.float32r
AF = mybir.ActivationFunctionType
ALU = mybir.AluOpType
AX = mybir.AxisListType

B, CH, H, W = 4, 64, 256, 256
HALF = 128
HALO = 18
SLAB = HALO + HALF + HALO  # 164
WP = W + 8                 # 264, data at cols X0..X0+255
X0 = 4
R0 = HALO                  # slab row of image row 0

EPS_CORR = 1e-6
EPS_IN = 1e-5

ROWS_F1 = (18, 162)
ROWS_T1 = (18, 161)
ROWS_FEAT = (18, 160)
ROWS_F1S = (18, 159)
ROWS_CORR = (18, 156)
ROWS_E1 = (18, 155)
ROWS_E2 = (18, 153)
ROWS_E3 = (18, 149)
ROWS_HEAD = (18, 148)
STAT_LO, STAT_HI = 18, 145

GROUPS = [[0, 1], [2, 3], [4, 5], [6, 7]]
TAPS = [(dy - 1, dx - 1) for dy in range(3) for dx in range(3)]


def _gauss1d():
    x = np.arange(7, dtype=np.float32) - 3.0
    g = np.exp(-(x ** 2) / (2.0 * 1.5 ** 2))
    return (g / g.sum()).astype(np.float32)


GG = _gauss1d()


# ---------------------------------------------------------------- host prep
def _tapT(w):
    """[O, I, 3, 3] -> [9, I, O] tap-major stationary layout."""
    o, i, _, _ = w.shape
    return np.ascontiguousarray(w.transpose(2, 3, 1, 0).reshape(9, i, o))


def _disp_vmat(rin_lo, rin_hi, ro_lo, ro_hi, clamp_lo):
    nin = rin_hi - rin_lo + 1
    nout = ro_hi - ro_lo + 1
    m = np.zeros((2 * nin, 2 * nout), np.float32)
    for ro in range(ro_lo, ro_hi + 1):
        for d in range(7):
            ri = ro + d - 3
            if clamp_lo is not None:
                ri = max(ri, clamp_lo)
            assert rin_lo <= ri <= rin_hi, (ro, d, ri)
            for c in range(2):
                m[2 * (ri - rin_lo) + c, 2 * (ro - ro_lo) + c] += GG[d]
    return m


def _prep_core_inputs(inp, core):
    b, h = core // 2, core % 2
    f1 = np.asarray(inp["feat1"][b], np.float32)
    f2 = np.asarray(inp["feat2"][b], np.float32)
    pre_w = np.asarray(inp["pre_w"]); pre_b = np.asarray(inp["pre_b"])
    fc1_w = np.asarray(inp["fc1_w"]); fc1_g = np.asarray(inp["fc1_g"]); fc1_be = np.asarray(inp["fc1_be"])
    fc2_w = np.asarray(inp["fc2_w"]); fc2_b = np.asarray(inp["fc2_b"])
    e1_w = np.asarray(inp["e1_w"]); e1_g = np.asarray(inp["e1_g"]); e1_be = np.asarray(inp["e1_be"])
    e2_w = np.asarray(inp["e2_w"]); e2_g = np.asarray(inp["e2_g"]); e2_be = np.asarray(inp["e2_be"])
    e3_w = np.asarray(inp["e3_w"]); e3_g = np.asarray(inp["e3_g"]); e3_be = np.asarray(inp["e3_be"])
    head_w = np.asarray(inp["head_w"]); head_b = np.asarray(inp["head_b"])

    if h == 1:
        f1 = f1[:, ::-1, :]
        f2 = f2[:, ::-1, :]
        flip = lambda w: w[:, :, ::-1, :]
        pre_w, fc1_w, fc2_w, e2_w, e3_w, head_w = map(
            flip, (pre_w, fc1_w, fc2_w, e2_w, e3_w, head_w))
        e1_w = flip(e1_w).copy()
        perm = np.array([(6 - i) * 7 + j for i in range(7) for j in range(7)])
        e1_w[:, 64:113] = e1_w[:, 64 + perm]

    def slab(x):
        s = np.zeros((CH, SLAB, WP), np.float32)
        s[:, R0:R0 + 146, X0:X0 + W] = x[:, 0:146, :]
        return s

    m0 = np.zeros((113, 1), np.float32); m0[0:64] = 1.0
    m1 = np.zeros((113, 1), np.float32)
    m1[64:113] = 1.0

    bd = np.stack([g * np.eye(64, dtype=np.float32) for g in GG])
    bdv = np.zeros((4, 128, 64), np.float32)
    for t in range(4):
        bdv[t, 0:64] = GG[2 * t] * np.eye(64, dtype=np.float32)
        if 2 * t + 1 < 7:
            bdv[t, 64:128] = GG[2 * t + 1] * np.eye(64, dtype=np.float32)
    bh128 = np.stack([g * np.eye(128, dtype=np.float32) for g in GG])
    bh112 = np.stack([g * np.eye(112, dtype=np.float32) for g in GG])

    bmv0 = _disp_vmat(18, 76, 18, 73, 18)
    bmv1 = _disp_vmat(71, 132, 74, 129, None)
    bmv2 = _disp_vmat(127, 148, 130, 145, None)

    # selector stationaries assembling lc = (A + B_shift - 2C)/64 in one psum:
    # C: 49 matmuls [128->98] with value -2/64
    cmat = np.zeros((49, 128, 98), np.float32)
    for k in range(49):
        cmat[k, 0:64, 2 * k] = -2.0 / 64.0
        cmat[k, 64:128, 2 * k + 1] = -2.0 / 64.0
    # B: 7 matmuls (one per horizontal shift j) [8 B-rows -> 98]
    bsel = np.zeros((7, 8, 98), np.float32)
    for i in range(7):
        for j in range(7):
            for q in range(2):
                bsel[j, i + q, 2 * (i * 7 + j) + q] = 1.0 / 64.0
    # A: 1 matmul [2 A-rows -> 98]
    asel = np.zeros((2, 98), np.float32)
    for k in range(49):
        for q in range(2):
            asel[q, 2 * k + q] = 1.0 / 64.0
    bd2 = np.zeros((128, 2), np.float32)
    bd2[0:64, 0] = 1.0
    bd2[64:128, 1] = 1.0
    zeros = np.zeros((128, 7 * WP), np.float32)

    d = {
        "feat1s": slab(f1), "feat2s": slab(f2),
        "preT": _tapT(pre_w), "pre_b": pre_b.reshape(64, 1),
        "fc1T": _tapT(fc1_w),
        "fc1_g": fc1_g.reshape(128, 1), "fc1_be": fc1_be.reshape(128, 1),
        "fc2T": _tapT(fc2_w), "fc2_b": fc2_b.reshape(64, 1),
        "e1T": _tapT(e1_w),
        "e1_g": e1_g.reshape(64, 1), "e1_be": e1_be.reshape(64, 1),
        "e2T": _tapT(e2_w), "e2_g": e2_g.reshape(32, 1), "e2_be": e2_be.reshape(32, 1),
        "e3T": _tapT(e3_w), "e3_g": e3_g.reshape(16, 1), "e3_be": e3_be.reshape(16, 1),
        "headT": _tapT(head_w), "head_b": head_b.reshape(2, 1),
        "m0": m0, "m1": m1,
        "bdv": bdv, "bd": bd, "bh128": bh128, "bh112": bh112, "cmat": cmat,
        "bsel": bsel, "asel": asel, "bd2": bd2, "zeros": zeros,
        "bmv0": bmv0, "bmv1": bmv1, "bmv2": bmv2,
    }
    return {k: np.ascontiguousarray(v, np.float32) for k, v in d.items()}


# ------------------------------------------------------------- wait fixer
# walrus in this container rejects instructions carrying more than a couple of
# sync waits; hoist excess waits onto single-wait NoOps in the same engine
# stream just before the instruction.
_SPLIT = {"InstDrain": 1, "InstMatmult": 0, "InstDMACopy": 1}
_SPLIT_DEFAULT = 1


def _fix_waits(nc):
    for fb in nc.m.functions[0].blocks:
        il = fb.instructions
        i = 0
        while i < len(il):
            inst = il[i]
            si = inst.sync_info
            mw = _SPLIT.get(type(inst).__name__, _SPLIT_DEFAULT)
            if si is not None and len(si.on_wait) > mw:
                ws = list(si.on_wait)
                si.on_wait = ws[:mw]
                inst.sync_info = si
                for j, wt in enumerate(ws[mw:]):
                    il.insert(i, mybir.InstNoOp(
                        name=f"{inst.name}-dw{j}", ins=[], outs=[],
                        engine=inst.engine, bass_nofuse=True,
                        sync_info=bass_rust.SyncInfo(on_wait=[wt], on_update=[])))
                    i += 1
            i += 1


def _pair_ap(buf, r):
    """Rows (r, r+1) of a [C, SLAB, WP] buf as a row-pair [2*64, WP] view."""
    return buf[:, r:r + 2, :].rearrange("c q x -> q c x")


# ------------------------------------------------------------- build
def build_module(collectives=True, upto=99):
    nc = bass.Bass(num_devices=8)

    def P(name, shape, dt=F32R):
        return nc.declare_dram_parameter(name, list(shape), dt, isOutput=False)

    f1in = P("feat1s", (CH, SLAB, WP))
    f2in = P("feat2s", (CH, SLAB, WP))
    preT = P("preT", (9, 64, 64)); pre_b = P("pre_b", (64, 1), F32)
    fc1T = P("fc1T", (9, 128, 128))
    fc1_g = P("fc1_g", (128, 1), F32); fc1_be = P("fc1_be", (128, 1), F32)
    fc2T = P("fc2T", (9, 128, 64)); fc2_b = P("fc2_b", (64, 1), F32)
    e1T = P("e1T", (9, 113, 64))
    e1_g = P("e1_g", (64, 1), F32); e1_be = P("e1_be", (64, 1), F32)
    e2T = P("e2T", (9, 64, 32))
    e2_g = P("e2_g", (32, 1), F32); e2_be = P("e2_be", (32, 1), F32)
    e3T = P("e3T", (9, 32, 16))
    e3_g = P("e3_g", (16, 1), F32); e3_be = P("e3_be", (16, 1), F32)
    headT = P("headT", (9, 16, 2)); head_b = P("head_b", (2, 1), F32)
    m0p = P("m0", (113, 1), F32); m1p = P("m1", (113, 1), F32)
    bdvp = P("bdv", (4, 128, 64)); bdp = P("bd", (7, 64, 64))
    bh128p = P("bh128", (7, 128, 128)); bh112p = P("bh112", (7, 112, 112))
    bmv0p = P("bmv0", (118, 112)); bmv1p = P("bmv1", (124, 112))
    bmv2p = P("bmv2", (44, 32))
    cmatp = P("cmat", (49, 128, 98))
    bselp = P("bsel", (7, 8, 98))
    aselp = P("asel", (2, 98))
    bd2p = P("bd2", (128, 2))
    zerop = P("zeros", (128, 7 * WP))

    out = nc.declare_dram_parameter("out", [2, HALF, W], F32, isOutput=True)

    f1b = nc.dram_tensor("f1b", [64, SLAB, WP], F32R)
    f2b = nc.dram_tensor("f2b", [64, SLAB, WP], F32R)
    t1b = nc.dram_tensor("t1b", [128, SLAB, WP], F32R)
    featb = nc.dram_tensor("featb", [64, SLAB, WP], F32R)
    f1sb = nc.dram_tensor("f1sb", [64, SLAB, WP], F32R)
    lcb = nc.dram_tensor("lcb", [49, SLAB, WP], F32R)
    bb = nc.dram_tensor("bb", [1, SLAB, WP], F32R)
    e1b = nc.dram_tensor("e1b", [64, SLAB, WP], F32R)
    e2b = nc.dram_tensor("e2b", [32, SLAB, WP], F32R)
    e3b = nc.dram_tensor("e3b", [16, SLAB, WP], F32R)
    headbuf = nc.dram_tensor("headbuf", [2, SLAB, WP], F32R)

    ar1i = nc.dram_tensor("ar1i", [1, 257], F32)
    ar1o = nc.dram_tensor("ar1o", [1, 257], F32)
    ar2i = nc.dram_tensor("ar2i", [1, 128], F32)
    ar2o = nc.dram_tensor("ar2o", [1, 128], F32)
    ar3i = nc.dram_tensor("ar3i", [1, 64], F32)
    ar3o = nc.dram_tensor("ar3o", [1, 64], F32)
    ar4i = nc.dram_tensor("ar4i", [1, 32], F32)
    ar4o = nc.dram_tensor("ar4o", [1, 32], F32)
    scr = nc.dram_tensor("scr", [1, 128], F32)

    with TileContext(nc) as tc, \
         tc.tile_pool(name="wpool", bufs=1) as wpool, \
         tc.tile_pool(name="cpool", bufs=1) as cpool, \
         tc.tile_pool(name="row", bufs=2) as rowp, \
         tc.tile_pool(name="big", bufs=4) as bigp, \
         tc.tile_pool(name="outp", bufs=3) as outp, \
         tc.tile_pool(name="stat", bufs=1) as statp, \
         tc.tile_pool(name="ps", bufs=4, space="PSUM") as psp, \
         tc.tile_pool(name="ps2", bufs=2, space="PSUM") as psp2:

        def wtile(src, shape, name, dt=F32R):
            t = wpool.tile(shape, dt, name=name)
            nc.sync.dma_start(out=t[:], in_=src)
            return t

        rr3 = lambda p: p[:, :, :].rearrange("t i o -> i t o")
        w_pre = wtile(rr3(preT), [64, 9 * 64], "w_pre")
        w_fc1 = wtile(rr3(fc1T), [128, 9 * 128], "w_fc1")
        w_fc2 = wtile(rr3(fc2T), [128, 9 * 64], "w_fc2")
        w_e1 = wtile(rr3(e1T), [113, 9 * 64], "w_e1")
        w_e2 = wtile(rr3(e2T), [64, 9 * 32], "w_e2")
        w_e3 = wtile(rr3(e3T), [32, 9 * 16], "w_e3")
        w_head = wtile(rr3(headT), [16, 9 * 2], "w_head")
        w_bdv = wtile(rr3(bdvp), [128, 4 * 64], "w_bdv")
        w_bd = wtile(rr3(bdp), [64, 7 * 64], "w_bd")
        w_bh128 = wtile(rr3(bh128p), [128, 7 * 128], "w_bh128")
        w_bh112 = wtile(rr3(bh112p), [112, 7 * 112], "w_bh112")
        w_bmv0 = wtile(bmv0p[:, :], [118, 112], "w_bmv0")
        w_bmv1 = wtile(bmv1p[:, :], [124, 112], "w_bmv1")
        w_bmv2 = wtile(bmv2p[:, :], [44, 32], "w_bmv2")
        w_cm = wtile(rr3(cmatp), [128, 49 * 98], "w_cm")
        w_bsel = wtile(bselp[:, :, :].rearrange("j u o -> u j o"), [8, 7 * 98], "w_bsel")
        w_asel = wtile(aselp[:, :], [2, 98], "w_asel")

        c_pre_b = wtile(pre_b[:, :], [64, 1], "c_pre_b", F32)
        c_fc1_g = wtile(fc1_g[:, :], [128, 1], "c_fc1_g", F32)
        c_fc1_be = wtile(fc1_be[:, :], [128, 1], "c_fc1_be", F32)
        c_fc2_b = wtile(fc2_b[:, :], [64, 1], "c_fc2_b", F32)
        c_e1_g = wtile(e1_g[:, :], [64, 1], "c_e1_g", F32)
        c_e1_be = wtile(e1_be[:, :], [64, 1], "c_e1_be", F32)
        c_e2_g = wtile(e2_g[:, :], [32, 1], "c_e2_g", F32)
        c_e2_be = wtile(e2_be[:, :], [32, 1], "c_e2_be", F32)
        c_e3_g = wtile(e3_g[:, :], [16, 1], "c_e3_g", F32)
        c_e3_be = wtile(e3_be[:, :], [16, 1], "c_e3_be", F32)
        c_head_b = wtile(head_b[:, :], [2, 1], "c_head_b", F32)
        c_m0 = wtile(m0p[:, :], [113, 1], "c_m0", F32)
        c_m1 = wtile(m1p[:, :], [113, 1], "c_m1", F32)

        c_bd2 = wtile(bd2p[:, :], [128, 2], "c_bd2")
        zt = wtile(zerop[:, :], [128, 7 * WP], "zt")

        for buf, c in [(f1b, 64), (f2b, 64), (t1b, 128), (featb, 64), (f1sb, 64),
                       (lcb, 49), (bb, 1), (e1b, 64), (e2b, 32),
                       (e3b, 16), (headbuf, 2)]:
            nc.sync.dma_start(out=buf[:, 11:18, :], in_=zt[:c, :7 * WP])
            nc.sync.dma_start(out=buf[:, :, 0:X0], in_=zt[:c, :SLAB * X0])
            nc.sync.dma_start(out=buf[:, :, WP - 4:WP], in_=zt[:c, :SLAB * 4])
        nc.sync.dma_start(out=f1b[:, 163:164, :], in_=zt[:64, :WP])

        st_fc1_s = statp.tile([128, 128], F32, name="st_fc1_s")
        st_fc1_q = statp.tile([128, 128], F32, name="st_fc1_q")
        st_e1_s = statp.tile([64, 128], F32, name="st_e1_s")
        st_e1_q = statp.tile([64, 128], F32, name="st_e1_q")
        st_e2_s = statp.tile([32, 128], F32, name="st_e2_s")
        st_e2_q = statp.tile([32, 128], F32, name="st_e2_q")
        st_e3_s = statp.tile([16, 128], F32, name="st_e3_s")
        st_e3_q = statp.tile([16, 128], F32, name="st_e3_q")
        st_c = statp.tile([98, 80], F32, name="st_c")
        for t in (st_fc1_s, st_fc1_q, st_e1_s, st_e1_q, st_e2_s, st_e2_q,
                  st_e3_s, st_e3_q, st_c):
            nc.vector.memset(t[:], 0.0)

        # ---------------- generic 3x3 conv pass ---------------------------
        def conv_pass(src_bufs, dst_buf, w_sb, cin, cout, dil, rows, tag,
                      bias=None, stats=None, norm=None):
            lo, hi = rows
            win = {}

            def load(r):
                t = rowp.tile([cin, WP], F32R, name=f"{tag}_in", tag="cin",
                              bufs=12)
                p = 0
                for bsrc, c in src_bufs:
                    nc.sync.dma_start(out=t[p:p + c, :], in_=bsrc[:, r, :])
                    p += c
                if norm is not None and r >= 18:
                    nc.scalar.activation(t[:, X0:X0 + W], t[:, X0:X0 + W], AF.Prelu,
                                         bias=norm[1], scale=norm[0], alpha=0.2)
                win[r] = t

            for r in range(lo - dil, lo + dil):
                load(r)
            for r in range(lo, hi + 1):
                load(r + dil)
                ps = psp.tile([cout, W], F32, name=f"{tag}_ps", tag="ps")
                for ti, (dy, dx) in enumerate(TAPS):
                    nc.tensor.matmul(
                        ps[:], w_sb[:, ti * cout:(ti + 1) * cout],
                        win[r + dy * dil][:, X0 + dx * dil:X0 + dx * dil + W],
                        start=(ti == 0), stop=(ti == 8))
                ot = outp.tile([cout, W], F32R, name=f"{tag}_o", tag="cout")
                acc = None
                if stats is not None and STAT_LO <= r <= STAT_HI:
                    acc = stats[0][:, r - STAT_LO:r - STAT_LO + 1]
                if bias is not None:
                    nc.scalar.activation(ot[:], ps[:], AF.Identity,
                                         bias=bias, accum_out=acc)
                else:
                    nc.scalar.activation(ot[:], ps[:], AF.Identity, accum_out=acc)
                if stats is not None and STAT_LO <= r <= STAT_HI:
                    sq = outp.tile([cout, W], F32, name=f"{tag}_sq", tag="csq")
                    nc.scalar.activation(
                        sq[:], ps[:], AF.Square,
                        accum_out=stats[1][:, r - STAT_LO:r - STAT_LO + 1])
                nc.sync.dma_start(out=dst_buf[:, r, X0:X0 + W], in_=ot[:])
                win.pop(r - dil, None)

        # ---------------- PASS 1: pre conv --------------------------------
        if upto >= 1:
            conv_pass([(f1in, 64)], f1b, w_pre, 64, 64, 1, ROWS_F1, "pre1",
                      bias=c_pre_b[:, 0:1])
            conv_pass([(f2in, 64)], f2b, w_pre, 64, 64, 1, ROWS_F1, "pre2",
                      bias=c_pre_b[:, 0:1])

        # ---------------- PASS 2: fc1 conv + stats ------------------------
        if upto >= 2:
            conv_pass([(f1b, 64), (f2b, 64)], t1b, w_fc1, 128, 128, 1, ROWS_T1, "fc1",
                      stats=(st_fc1_s, st_fc1_q))

        # ---------------- PASS 3: gaussian blur of f1, B ------------------
        if upto >= 3:
            fwin = {}

            def loadpair(r):
                t = rowp.tile([128, WP], F32R, name="f1p_in", tag="pin", bufs=11)
                nc.sync.dma_start(out=t[:], in_=_pair_ap(f1b, r))
                fwin[r] = t

            lo, hi = ROWS_F1S
            for r in range(lo, lo + 8):
                loadpair(r)
            vpair = None
            for r in range(lo, hi + 1):
                k = r + 3
                if k not in fwin and k <= 162:
                    loadpair(k)
                q = (r - lo) % 2
                if q == 0:
                    vpair = bigp.tile([128, WP], F32R, name="vpair", tag="bigA")
                    nc.vector.tensor_copy(vpair[:, 0:X0], zt[:, 0:X0])
                    nc.vector.tensor_copy(vpair[:, WP - 4:WP], zt[:, 0:4])
                psv = psp2.tile([64, W], F32, name="ps_v", tag="ps2")
                if r < lo + 3:
                    for d in range(7):
                        ri = max(r + d - 3, lo)
                        nc.tensor.matmul(psv[:], w_bd[:, d * 64:(d + 1) * 64],
                                         fwin[ri][0:64, X0:X0 + W],
                                         start=(d == 0), stop=(d == 6))
                else:
                    for t in range(4):
                        nc.tensor.matmul(psv[:], w_bdv[:, t * 64:(t + 1) * 64],
                                         fwin[r - 3 + 2 * t][:, X0:X0 + W],
                                         start=(t == 0), stop=(t == 3))
                nc.scalar.activation(vpair[q * 64:(q + 1) * 64, X0:X0 + W], psv[:],
                                     AF.Identity)
                if q == 1:
                    nc.vector.tensor_copy(vpair[:, 1:4],
                                          vpair[:, X0:X0 + 1].to_broadcast([128, 3]))
                    nc.vector.tensor_copy(vpair[:, WP - 4:WP - 1],
                                          vpair[:, X0 + W - 1:X0 + W].to_broadcast([128, 3]))
                    psh = psp.tile([128, W], F32, name="ps_h", tag="ps")
                    for j in range(7):
                        nc.tensor.matmul(psh[:], w_bh128[:, j * 128:(j + 1) * 128],
                                         vpair[:, 1 + j:1 + j + W],
                                         start=(j == 0), stop=(j == 6))
                    f1st = bigp.tile([128, W], F32R, name="f1st", tag="bigC", bufs=6)
                    nc.scalar.activation(f1st[:], psh[:], AF.Identity)
                    nc.sync.dma_start(
                        out=f1sb[:, r - 1:r + 1, X0:X0 + W].rearrange("c q x -> q c x"),
                        in_=f1st[:])
                    sqt = bigp.tile([128, W], F32R, name="sqB", tag="bigB")
                    nc.scalar.activation(sqt[:], psh[:], AF.Square)
                    psb = psp2.tile([2, W], F32, name="ps_b", tag="ps2")
                    nc.tensor.matmul(psb[:], c_bd2[:], sqt[:], start=True, stop=True)
                    bt = outp.tile([2, W], F32R, name="bt", tag="small")
                    nc.scalar.activation(bt[:], psb[:], AF.Identity)
                    nc.sync.dma_start(out=bb[0, r - 1:r + 1, X0:X0 + W], in_=bt[:])
                fwin.pop(r - 4, None)

        # ---------------- PASS 4: correlation -----------------------------
        if upto >= 4:
            swin = {}

            def loadspair(r):
                t = rowp.tile([128, WP], F32R, name="f1sp_in", tag="pin", bufs=11)
                nc.sync.dma_start(out=t[:], in_=_pair_ap(f1sb, r))
                swin[r] = t

            lo, hi = ROWS_CORR
            pair_starts = list(range(lo, 155, 2)) + [155]
            for r in range(lo - 3, lo + 4):
                loadspair(r)
            for pidx, r in enumerate(pair_starts):
                for rr in range(r - 3, r + 4):
                    if rr not in swin:
                        loadspair(rr)
                f2p = rowp.tile([128, WP], F32R, name="f2p", tag="aux", bufs=4)
                nc.sync.dma_start(out=f2p[:], in_=_pair_ap(f2b, r))
                sqa = bigp.tile([128, W], F32R, name="sqA", tag="bigB")
                nc.scalar.activation(sqa[:], f2p[:, X0:X0 + W], AF.Square)
                psa = psp2.tile([2, W], F32, name="ps_a", tag="ps2")
                nc.tensor.matmul(psa[:], c_bd2[:], sqa[:], start=True, stop=True)
                at = outp.tile([2, W], F32R, name="at", tag="small")
                nc.scalar.activation(at[:], psa[:], AF.Identity)
                bw8 = rowp.tile([8, WP], F32R, name="bw8", tag="aux", bufs=4)
                nc.sync.dma_start(out=bw8[:], in_=bb[0, r - 3:r + 5, :])
                psc = psp.tile([98, W], F32, name="ps_c", tag="ps")
                for i in range(7):
                    sp = swin[r + i - 3]
                    for j in range(7):
                        kk = i * 7 + j
                        prod = bigp.tile([128, W], F32R, name="prod", tag="bigC", bufs=6)
                        nc.vector.tensor_tensor(
                            prod[:], f2p[:, X0:X0 + W],
                            sp[:, X0 + j - 3:X0 + j - 3 + W], ALU.mult)
                        nc.tensor.matmul(psc[:], w_cm[:, kk * 98:(kk + 1) * 98],
                                         prod[:], start=(kk == 0), stop=False)
                for j in range(7):
                    nc.tensor.matmul(psc[:], w_bsel[:, j * 98:(j + 1) * 98],
                                     bw8[:, X0 + j - 3:X0 + j - 3 + W],
                                     start=False, stop=False)
                nc.tensor.matmul(psc[:], w_asel[:], at[:], start=False, stop=True)
                cred = outp.tile([98, 1], F32, name="cred", tag="small")
                nc.vector.tensor_reduce(cred[:], psc[:], AX.X, ALU.add)
                nc.vector.tensor_copy(st_c[:, pidx:pidx + 1], cred[:])
                ct = bigp.tile([98, W], F32R, name="ct", tag="bigA")
                nc.scalar.activation(ct[:], psc[:], AF.Identity)
                nc.sync.dma_start(out=lcb[:, r:r + 2, X0:X0 + W], in_=ct[:])
                for kk in (r - 3, r - 2):
                    swin.pop(kk, None)

        # ---------------- AR1 ---------------------------------------------
        if upto >= 5:
            fs = statp.tile([128, 1], F32, name="fs")
            fq = statp.tile([128, 1], F32, name="fq")
            nc.vector.tensor_reduce(fs[:], st_fc1_s[:], AX.X, ALU.add)
            nc.vector.tensor_reduce(fq[:], st_fc1_q[:], AX.X, ALU.add)
            credp = statp.tile([98, 1], F32, name="credp")
            nc.vector.tensor_reduce(credp[:], st_c[:, 0:64], AX.X, ALU.add)
            # C partial: flatten [98,1] -> [1,98] via DRAM, reduce to scalar
            nc.sync.dma_start(out=scr[0:1, 0:98], in_=credp[:])
            ctot_r = statp.tile([1, 98], F32, name="ctot_r")
            nc.sync.dma_start(out=ctot_r[:], in_=scr[0:1, 0:98])
            ctot = statp.tile([1, 1], F32, name="ctot")
            nc.vector.tensor_reduce(ctot[:], ctot_r[:], AX.X, ALU.add)
            # payload: [0:128) sum, [128:256) sumsq, 256 = lc total
            nc.sync.dma_start(out=ar1i[0:1, 0:128], in_=fs[:])
            nc.sync.dma_start(out=ar1i[0:1, 128:256], in_=fq[:])
            nc.sync.dma_start(out=ar1i[0:1, 256:257], in_=ctot[:])
            if collectives:
                nc.gpsimd.collective_compute("AllReduce", ALU.add, replica_groups=GROUPS,
                                             ins=[ar1i[:, :]], outs=[ar1o[:, :]])
            else:
                nc.sync.dma_start(out=ar1o[:, :], in_=ar1i[:, :])
            fc1s = statp.tile([128, 1], F32, name="fc1s")
            fc1q = statp.tile([128, 1], F32, name="fc1q")
            nc.sync.dma_start(out=fc1s[:], in_=ar1o[0:1, 0:128])
            nc.sync.dma_start(out=fc1q[:], in_=ar1o[0:1, 128:256])
            sc = statp.tile([1, 1], F32, name="sc")
            nc.sync.dma_start(out=sc[:], in_=ar1o[0:1, 256:257])

            def norm_params(ssum, sq, gam, bet, n, c, nm):
                mu = statp.tile([c, 1], F32, name=f"mu_{nm}")
                nc.vector.tensor_scalar_mul(mu[:], ssum[:], 1.0 / n)
                var = statp.tile([c, 1], F32, name=f"var_{nm}")
                nc.vector.tensor_scalar_mul(var[:], sq[:], 1.0 / n)
                mu2 = statp.tile([c, 1], F32, name=f"mu2_{nm}")
                nc.vector.tensor_tensor(mu2[:], mu[:], mu[:], ALU.mult)
                nc.vector.tensor_tensor(var[:], var[:], mu2[:], ALU.subtract)
                nc.vector.tensor_scalar_add(var[:], var[:], EPS_IN)
                sd = statp.tile([c, 1], F32, name=f"sd_{nm}")
                nc.scalar.activation(sd[:], var[:], AF.Sqrt)
                rstd = statp.tile([c, 1], F32, name=f"rstd_{nm}")
                nc.vector.reciprocal(rstd[:], sd[:])
                s_ = statp.tile([c, 1], F32, name=f"s_{nm}")
                nc.vector.tensor_tensor(s_[:], gam[:], rstd[:], ALU.mult)
                b_ = statp.tile([c, 1], F32, name=f"b_{nm}")
                nc.vector.tensor_tensor(b_[:], mu[:], s_[:], ALU.mult)
                nc.vector.tensor_tensor(b_[:], bet[:], b_[:], ALU.subtract)
                return s_[:, 0:1], b_[:, 0:1]

            fc1_sn, fc1_bn = norm_params(fc1s, fc1q, c_fc1_g, c_fc1_be, 65536.0, 128, "fc1")

            # s = lcT / (49*H*W) + eps ; rs = 1/s ; scale e1 lc-channel weights
            sval = statp.tile([1, 1], F32, name="sval")
            nc.vector.tensor_scalar(sval[:], sc[0:1, 0:1], 1.0 / (49.0 * H * W), None,
                                    ALU.mult)
            nc.vector.tensor_scalar_add(sval[:], sval[:], EPS_CORR)
            rsv = statp.tile([1, 1], F32, name="rsv")
            nc.vector.reciprocal(rsv[:], sval[:])
            nc.sync.dma_start(out=scr[0:1, 100:101], in_=rsv[:])
            rb113 = statp.tile([113, 1], F32, name="rb113")
            nc.sync.dma_start(out=rb113[:],
                              in_=bass.AP(scr, 100, [[0, 113], [1, 1]]))
            fac = statp.tile([113, 1], F32, name="fac")
            nc.vector.tensor_tensor(fac[:], c_m1[:], rb113[:], ALU.mult)
            nc.vector.tensor_tensor(fac[:], fac[:], c_m0[:], ALU.add)
            nc.vector.tensor_scalar(w_e1[:], w_e1[:], fac[:, 0:1], None, ALU.mult)

        # ---------------- PASS 5: fc2 -------------------------------------
        if upto >= 6:
            conv_pass([(t1b, 128)], featb, w_fc2, 128, 64, 1, ROWS_FEAT, "fc2",
                      bias=c_fc2_b[:, 0:1], norm=(fc1_sn, fc1_bn))

        # ---------------- PASS 6: e1 --------------------------------------
        if upto >= 7:
            lo, hi = ROWS_E1
            ewin = {}

            def eload(r):
                t = rowp.tile([113, WP], F32R, name="e1_in", tag="cin", bufs=12)
                nc.sync.dma_start(out=t[0:64, :], in_=featb[:, r, :])
                nc.sync.dma_start(out=t[64:113, :], in_=lcb[:, r, :])
                ewin[r] = t

            for r in range(lo - 1, lo + 1):
                eload(r)
            for r in range(lo, hi + 1):
                eload(r + 1)
                ps = psp.tile([64, W], F32, name="e1_ps", tag="ps")
                for ti, (dy, dx) in enumerate(TAPS):
                    nc.tensor.matmul(ps[:], w_e1[:, ti * 64:(ti + 1) * 64],
                                     ewin[r + dy][:, X0 + dx:X0 + dx + W],
                                     start=(ti == 0), stop=(ti == 8))
                ot = outp.tile([64, W], F32R, name="e1_o", tag="cout")
                acc = st_e1_s[:, r - STAT_LO:r - STAT_LO + 1] if STAT_LO <= r <= STAT_HI else None
                nc.scalar.activation(ot[:], ps[:], AF.Identity, accum_out=acc)
                if STAT_LO <= r <= STAT_HI:
                    sq = outp.tile([64, W], F32, name="e1_sq", tag="csq")
                    nc.scalar.activation(sq[:], ps[:], AF.Square,
                                         accum_out=st_e1_q[:, r - STAT_LO:r - STAT_LO + 1])
                nc.sync.dma_start(out=e1b[:, r, X0:X0 + W], in_=ot[:])
                ewin.pop(r - 1, None)

        # ---- AR2 / e2 / AR3 / e3 / AR4 / head ---------------------------
        if upto >= 8:
            def stats_ar(st_s, st_q, c, ari, aro, nm):
                s1 = statp.tile([c, 1], F32, name=f"ss_{nm}")
                q1 = statp.tile([c, 1], F32, name=f"qq_{nm}")
                nc.vector.tensor_reduce(s1[:], st_s[:], AX.X, ALU.add)
                nc.vector.tensor_reduce(q1[:], st_q[:], AX.X, ALU.add)
                nc.sync.dma_start(out=ari[0:1, 0:c], in_=s1[:])
                nc.sync.dma_start(out=ari[0:1, c:2 * c], in_=q1[:])
                if collectives:
                    nc.gpsimd.collective_compute("AllReduce", ALU.add,
                                                 replica_groups=GROUPS,
                                                 ins=[ari[:, :]], outs=[aro[:, :]])
                else:
                    nc.sync.dma_start(out=aro[:, :], in_=ari[:, :])
                s2 = statp.tile([c, 1], F32, name=f"ss2_{nm}")
                q2 = statp.tile([c, 1], F32, name=f"qq2_{nm}")
                nc.sync.dma_start(out=s2[:], in_=aro[0:1, 0:c])
                nc.sync.dma_start(out=q2[:], in_=aro[0:1, c:2 * c])
                return s2, q2

            e1s2, e1q2 = stats_ar(st_e1_s, st_e1_q, 64, ar2i, ar2o, "e1")
            e1_sn, e1_bn = norm_params(e1s2, e1q2, c_e1_g, c_e1_be, 65536.0, 64, "e1")

            conv_pass([(e1b, 64)], e2b, w_e2, 64, 32, 2, ROWS_E2, "e2",
                      stats=(st_e2_s, st_e2_q), norm=(e1_sn, e1_bn))
            e2s2, e2q2 = stats_ar(st_e2_s, st_e2_q, 32, ar3i, ar3o, "e2")
            e2_sn, e2_bn = norm_params(e2s2, e2q2, c_e2_g, c_e2_be, 65536.0, 32, "e2")

            conv_pass([(e2b, 32)], e3b, w_e3, 32, 16, 4, ROWS_E3, "e3",
                      stats=(st_e3_s, st_e3_q), norm=(e2_sn, e2_bn))
            e3s2, e3q2 = stats_ar(st_e3_s, st_e3_q, 16, ar4i, ar4o, "e3")
            e3_sn, e3_bn = norm_params(e3s2, e3q2, c_e3_g, c_e3_be, 65536.0, 16, "e3")

            conv_pass([(e3b, 16)], headbuf, w_head, 16, 2, 1, ROWS_HEAD, "head",
                      bias=c_head_b[:, 0:1], norm=(e3_sn, e3_bn))

        # ---------------- PASS 10: disp blur ------------------------------
        if upto >= 9:
            blocks = [
                (w_bmv0, 118, 18, 59, 18, 56, 112),
                (w_bmv1, 124, 71, 62, 74, 56, 112),
                (w_bmv2, 44, 127, 22, 130, 16, 32),
            ]
            for wm, kp, rin0, nin, ro0, nout, mp in blocks:
                ht = bigp.tile([kp, WP], F32R, name="dh_in", tag="bigA")
                nc.sync.dma_start(
                    out=ht[:],
                    in_=headbuf[:, rin0:rin0 + nin, :].rearrange("c r x -> r c x"))
                psv = psp.tile([mp, WP], F32, name="dps_v", tag="ps")
                nc.tensor.matmul(psv[:], wm[:], ht[:], start=True, stop=True)
                vt = bigp.tile([mp, WP], F32R, name="dvt", tag="bigB")
                nc.scalar.activation(vt[:], psv[:], AF.Identity)
                nc.vector.tensor_copy(vt[:, 1:4], vt[:, X0:X0 + 1].to_broadcast([mp, 3]))
                nc.vector.tensor_copy(vt[:, WP - 4:WP - 1],
                                      vt[:, X0 + W - 1:X0 + W].to_broadcast([mp, 3]))
                psh = psp.tile([mp, W], F32, name="dps_h", tag="ps")
                for j in range(7):
                    nc.tensor.matmul(psh[:], w_bh112[0:mp, j * 112:j * 112 + mp],
                                     vt[:, 1 + j:1 + j + W], start=(j == 0), stop=(j == 6))
                ot = outp.tile([mp, W], F32, name="do", tag="small")
                nc.scalar.activation(ot[:], psh[:], AF.Identity)
                nc.sync.dma_start(
                    out=out[:, ro0 - R0:ro0 - R0 + nout, :].rearrange("c r x -> r c x"),
                    in_=ot[:])

    _fix_waits(nc)
    return nc


# ---------------------------------------------------------------- entry
_NC = None


def kernel(**inputs):
    global _NC
    if _NC is None:
        _NC = build_module()
    in_maps = [_prep_core_inputs(inputs, c) for c in range(8)]
    res = run_bass_kernel_spmd(_NC, in_maps, list(range(8)))
    disp = np.zeros((B, 2, H, W), np.float32)
    for c in range(8):
        b, h = c // 2, c % 2
        o = res.results[c]["out"]
        if h == 0:
            disp[b, :, 0:HALF, :] = o
        else:
            disp[b, :, HALF:H, :] = o[:, ::-1, :]
    return disp



# revision 19
# speedup vs baseline: 1.1716x; 1.1716x over previous
"""DispEstimator Trainium2 kernel: 8-core SPMD (batch x H-half sharding).

Core c handles sample b=c//2, vertical half h=c%2. Odd cores get a vertically
flipped view of their sample (dy-flipped weights, permuted lc-channel order
for e1), so every core runs identical code: the "top" slab edge is a true
image boundary, the "bottom" edge is interior halo. Instance-norm and
correlation-normalization statistics are made global via pairwise AllReduce
between the two cores of each sample.

Correlation: lc_k = (A + B_shift - 2*C_k)/64 with A = sum_c f2^2,
B = sum_c f1s^2, C_k = sum_c f2 * shift_k(f1s). C is computed on
DVE (products) + TensorE (pairwise block-ones reduction). A rides as an extra
e1 input channel (summed weights); B's contribution is folded into a 9x9
single-channel conv on B executed as extra accumulating matmuls in e1. The
1/(64*s) normalization is baked into the e1/B weight tiles after AllReduce.
"""
import sys

import numpy as np

if "/opt/trn_rl_repo" not in sys.path:
    sys.path.insert(0, "/opt/trn_rl_repo")

import bass_rust
import concourse.bass as bass
import concourse.mybir as mybir
from concourse.bass_utils import run_bass_kernel_spmd
from concourse.tile import TileContext

F32 = mybir.dt.float32
F32R = mybir.dt.float32r
AF = mybir.ActivationFunctionType
ALU = mybir.AluOpType
AX = mybir.AxisListType

B, CH, H, W = 4, 64, 256, 256
HALF = 128
HALO = 18
SLAB = HALO + HALF + HALO  # 164
WP = W + 8                 # 264, data at cols X0..X0+255
X0 = 4
R0 = HALO                  # slab row of image row 0

EPS_CORR = 1e-6
EPS_IN = 1e-5

ROWS_F1 = (18, 162)
ROWS_T1 = (18, 161)
ROWS_FEAT = (18, 160)
ROWS_F1S = (18, 159)
ROWS_CORR = (18, 156)
ROWS_E1 = (18, 155)
ROWS_E2 = (18, 153)
ROWS_E3 = (18, 149)
ROWS_HEAD = (18, 148)
STAT_LO, STAT_HI = 18, 145

GROUPS = [[0, 1], [2, 3], [4, 5], [6, 7]]
TAPS = [(dy - 1, dx - 1) for dy in range(3) for dx in range(3)]


def _gauss1d():
    x = np.arange(7, dtype=np.float32) - 3.0
    g = np.exp(-(x ** 2) / (2.0 * 1.5 ** 2))
    return (g / g.sum()).astype(np.float32)


GG = _gauss1d()


# ---------------------------------------------------------------- host prep
def _tapT(w):
    """[O, I, 3, 3] -> [9, I, O] tap-major stationary layout."""
    o, i, _, _ = w.shape
    return np.ascontiguousarray(w.transpose(2, 3, 1, 0).reshape(9, i, o))


def _disp_vmat(rin_lo, rin_hi, ro_lo, ro_hi, clamp_lo):
    nin = rin_hi - rin_lo + 1
    nout = ro_hi - ro_lo + 1
    m = np.zeros((2 * nin, 2 * nout), np.float32)
    for ro in range(ro_lo, ro_hi + 1):
        for d in range(7):
            ri = ro + d - 3
            if clamp_lo is not None:
                ri = max(ri, clamp_lo)
            assert rin_lo <= ri <= rin_hi, (ro, d, ri)
            for c in range(2):
                m[2 * (ri - rin_lo) + c, 2 * (ro - ro_lo) + c] += GG[d]
    return m


def _prep_core_inputs(inp, core):
    b, h = core // 2, core % 2
    f1 = np.asarray(inp["feat1"][b], np.float32)
    f2 = np.asarray(inp["feat2"][b], np.float32)
    pre_w = np.asarray(inp["pre_w"]); pre_b = np.asarray(inp["pre_b"])
    fc1_w = np.asarray(inp["fc1_w"]); fc1_g = np.asarray(inp["fc1_g"]); fc1_be = np.asarray(inp["fc1_be"])
    fc2_w = np.asarray(inp["fc2_w"]); fc2_b = np.asarray(inp["fc2_b"])
    e1_w = np.asarray(inp["e1_w"]); e1_g = np.asarray(inp["e1_g"]); e1_be = np.asarray(inp["e1_be"])
    e2_w = np.asarray(inp["e2_w"]); e2_g = np.asarray(inp["e2_g"]); e2_be = np.asarray(inp["e2_be"])
    e3_w = np.asarray(inp["e3_w"]); e3_g = np.asarray(inp["e3_g"]); e3_be = np.asarray(inp["e3_be"])
    head_w = np.asarray(inp["head_w"]); head_b = np.asarray(inp["head_b"])

    if h == 1:
        f1 = f1[:, ::-1, :]
        f2 = f2[:, ::-1, :]
        flip = lambda w: w[:, :, ::-1, :]
        pre_w, fc1_w, fc2_w, e2_w, e3_w, head_w = map(
            flip, (pre_w, fc1_w, fc2_w, e2_w, e3_w, head_w))
        e1_w = flip(e1_w).copy()
        perm = np.array([(6 - i) * 7 + j for i in range(7) for j in range(7)])
        e1_w[:, 64:113] = e1_w[:, 64 + perm]

    def slab(x):
        s = np.zeros((CH, SLAB, WP), np.float32)
        s[:, R0:R0 + 146, X0:X0 + W] = x[:, 0:146, :]
        return s

    m0 = np.zeros((113, 1), np.float32); m0[0:64] = 1.0
    m1 = np.zeros((113, 1), np.float32)
    m1[64:113] = 1.0

    bd = np.stack([g * np.eye(64, dtype=np.float32) for g in GG])
    bdv = np.zeros((4, 128, 64), np.float32)
    for t in range(4):
        bdv[t, 0:64] = GG[2 * t] * np.eye(64, dtype=np.float32)
        if 2 * t + 1 < 7:
            bdv[t, 64:128] = GG[2 * t + 1] * np.eye(64, dtype=np.float32)
    bh128 = np.stack([g * np.eye(128, dtype=np.float32) for g in GG])
    bh112 = np.stack([g * np.eye(112, dtype=np.float32) for g in GG])

    bmv0 = _disp_vmat(18, 76, 18, 73, 18)
    bmv1 = _disp_vmat(71, 132, 74, 129, None)
    bmv2 = _disp_vmat(127, 148, 130, 145, None)

    # selector stationaries assembling lc = (A + B_shift - 2C)/64 in one psum:
    # C: 49 matmuls [128->98] with value -2/64
    cmat = np.zeros((49, 128, 98), np.float32)
    for k in range(49):
        cmat[k, 0:64, 2 * k] = -2.0 / 64.0
        cmat[k, 64:128, 2 * k + 1] = -2.0 / 64.0
    # B: 7 matmuls (one per horizontal shift j) [8 B-rows -> 98]
    bsel = np.zeros((7, 8, 98), np.float32)
    for i in range(7):
        for j in range(7):
            for q in range(2):
                bsel[j, i + q, 2 * (i * 7 + j) + q] = 1.0 / 64.0
    # A: 1 matmul [2 A-rows -> 98]
    asel = np.zeros((2, 98), np.float32)
    for k in range(49):
        for q in range(2):
            asel[q, 2 * k + q] = 1.0 / 64.0
    bd2 = np.zeros((128, 2), np.float32)
    bd2[0:64, 0] = 1.0
    bd2[64:128, 1] = 1.0
    zeros = np.zeros((128, WP), np.float32)

    preT = _tapT(pre_w)
    preT2 = np.zeros((9, 128, 128), np.float32)
    preT2[:, 0:64, 0:64] = preT
    preT2[:, 64:128, 64:128] = preT
    d = {
        "feat1s": slab(f1), "feat2s": slab(f2),
        "preT": preT2, "pre_b": np.concatenate([pre_b, pre_b]).reshape(128, 1),
        "fc1T": _tapT(fc1_w),
        "fc1_g": fc1_g.reshape(128, 1), "fc1_be": fc1_be.reshape(128, 1),
        "fc2T": _tapT(fc2_w), "fc2_b": fc2_b.reshape(64, 1),
        "e1T": _tapT(e1_w),
        "e1_g": e1_g.reshape(64, 1), "e1_be": e1_be.reshape(64, 1),
        "e2T": _tapT(e2_w), "e2_g": e2_g.reshape(32, 1), "e2_be": e2_be.reshape(32, 1),
        "e3T": _tapT(e3_w), "e3_g": e3_g.reshape(16, 1), "e3_be": e3_be.reshape(16, 1),
        "headT": _tapT(head_w), "head_b": head_b.reshape(2, 1),
        "m0": m0, "m1": m1,
        "bdv": bdv, "bd": bd, "bh128": bh128, "bh112": bh112, "cmat": cmat,
        "bsel": bsel, "asel": asel, "bd2": bd2, "zeros": zeros,
        "bmv0": bmv0, "bmv1": bmv1, "bmv2": bmv2,
    }
    return {k: np.ascontiguousarray(v, np.float32) for k, v in d.items()}


# ------------------------------------------------------------- wait fixer
# walrus in this container rejects instructions carrying more than a couple of
# sync waits; hoist excess waits onto single-wait NoOps in the same engine
# stream just before the instruction.
_SPLIT = {"InstDrain": 1, "InstMatmult": 0, "InstDMACopy": 1}
_SPLIT_DEFAULT = 1


def _fix_waits(nc):
    for fb in nc.m.functions[0].blocks:
        il = fb.instructions
        i = 0
        while i < len(il):
            inst = il[i]
            si = inst.sync_info
            mw = _SPLIT.get(type(inst).__name__, _SPLIT_DEFAULT)
            if si is not None and len(si.on_wait) > mw:
                ws = list(si.on_wait)
                si.on_wait = ws[:mw]
                inst.sync_info = si
                for j, wt in enumerate(ws[mw:]):
                    il.insert(i, mybir.InstNoOp(
                        name=f"{inst.name}-dw{j}", ins=[], outs=[],
                        engine=inst.engine, bass_nofuse=True,
                        sync_info=bass_rust.SyncInfo(on_wait=[wt], on_update=[])))
                    i += 1
            i += 1


def _pair_ap(buf, r):
    """Rows (r, r+1) of a [C, SLAB, WP] buf as a row-pair [2*64, WP] view."""
    return buf[:, r:r + 2, :].rearrange("c q x -> q c x")


def _sp4(t, d0, d1, d2, d3):
    """4-dim AP over a 2-D sbuf tile: partition split (d0,d1), col split (d2,d3)."""
    a = t[:, :]
    pc = a.ap[0][0]
    return bass.AP(a.tensor, a.offset,
                   [[pc * d1, d0], [pc, d1], [d3, d2], [1, d3]])


# ------------------------------------------------------------- build
def build_module(collectives=True, upto=99):
    nc = bass.Bass(num_devices=8)

    def P(name, shape, dt=F32R):
        return nc.declare_dram_parameter(name, list(shape), dt, isOutput=False)

    f1in = P("feat1s", (CH, SLAB, WP))
    f2in = P("feat2s", (CH, SLAB, WP))
    preT = P("preT", (9, 128, 128)); pre_b = P("pre_b", (128, 1), F32)
    fc1T = P("fc1T", (9, 128, 128))
    fc1_g = P("fc1_g", (128, 1), F32); fc1_be = P("fc1_be", (128, 1), F32)
    fc2T = P("fc2T", (9, 128, 64)); fc2_b = P("fc2_b", (64, 1), F32)
    e1T = P("e1T", (9, 113, 64))
    e1_g = P("e1_g", (64, 1), F32); e1_be = P("e1_be", (64, 1), F32)
    e2T = P("e2T", (9, 64, 32))
    e2_g = P("e2_g", (32, 1), F32); e2_be = P("e2_be", (32, 1), F32)
    e3T = P("e3T", (9, 32, 16))
    e3_g = P("e3_g", (16, 1), F32); e3_be = P("e3_be", (16, 1), F32)
    headT = P("headT", (9, 16, 2)); head_b = P("head_b", (2, 1), F32)
    m0p = P("m0", (113, 1), F32); m1p = P("m1", (113, 1), F32)
    bdvp = P("bdv", (4, 128, 64)); bdp = P("bd", (7, 64, 64))
    bh128p = P("bh128", (7, 128, 128)); bh112p = P("bh112", (7, 112, 112))
    bmv0p = P("bmv0", (118, 112)); bmv1p = P("bmv1", (124, 112))
    bmv2p = P("bmv2", (44, 32))
    cmatp = P("cmat", (49, 128, 98))
    bselp = P("bsel", (7, 8, 98))
    aselp = P("asel", (2, 98))
    bd2p = P("bd2", (128, 2))
    zerop = P("zeros", (128, WP))

    out = nc.declare_dram_parameter("out", [2, HALF, W], F32, isOutput=True)

    f1b = nc.dram_tensor("f1b", [64, SLAB, WP], F32R)
    f2b = nc.dram_tensor("f2b", [64, SLAB, WP], F32R)
    t1b = nc.dram_tensor("t1b", [128, SLAB, WP], F32R)
    featb = nc.dram_tensor("featb", [64, SLAB, WP], F32R)
    f1sb = nc.dram_tensor("f1sb", [64, SLAB, WP], F32R)
    lcb = nc.dram_tensor("lcb", [49, SLAB, WP], F32R)
    bb = nc.dram_tensor("bb", [1, SLAB, WP], F32R)
    e1b = nc.dram_tensor("e1b", [64, SLAB, WP], F32R)
    e2b = nc.dram_tensor("e2b", [32, SLAB, WP], F32R)
    e3b = nc.dram_tensor("e3b", [16, SLAB, WP], F32R)
    headbuf = nc.dram_tensor("headbuf", [2, SLAB, WP], F32R)

    ar1i = nc.dram_tensor("ar1i", [1, 257], F32)
    ar1o = nc.dram_tensor("ar1o", [1, 257], F32)
    ar2i = nc.dram_tensor("ar2i", [1, 128], F32)
    ar2o = nc.dram_tensor("ar2o", [1, 128], F32)
    ar3i = nc.dram_tensor("ar3i", [1, 64], F32)
    ar3o = nc.dram_tensor("ar3o", [1, 64], F32)
    ar4i = nc.dram_tensor("ar4i", [1, 32], F32)
    ar4o = nc.dram_tensor("ar4o", [1, 32], F32)
    scr = nc.dram_tensor("scr", [1, 128], F32)

    with TileContext(nc) as tc, \
         tc.tile_pool(name="wpool", bufs=1) as wpool, \
         tc.tile_pool(name="cpool", bufs=1) as cpool, \
         tc.tile_pool(name="row", bufs=2) as rowp, \
         tc.tile_pool(name="big", bufs=4) as bigp, \
         tc.tile_pool(name="outp", bufs=3) as outp, \
         tc.tile_pool(name="stat", bufs=1) as statp, \
         tc.tile_pool(name="ps", bufs=4, space="PSUM") as psp, \
         tc.tile_pool(name="ps2", bufs=2, space="PSUM") as psp2:

        def wtile(src, shape, name, dt=F32R):
            t = wpool.tile(shape, dt, name=name)
            nc.sync.dma_start(out=t[:], in_=src)
            return t

        rr3 = lambda p: p[:, :, :].rearrange("t i o -> i t o")
        w_pre = wtile(rr3(preT), [128, 9 * 128], "w_pre")
        w_fc1 = wtile(rr3(fc1T), [128, 9 * 128], "w_fc1")
        w_fc2 = wtile(rr3(fc2T), [128, 9 * 64], "w_fc2")
        w_e1 = wtile(rr3(e1T), [113, 9 * 64], "w_e1")
        w_e2 = wtile(rr3(e2T), [64, 9 * 32], "w_e2")
        w_e3 = wtile(rr3(e3T), [32, 9 * 16], "w_e3")
        w_head = wtile(rr3(headT), [16, 9 * 2], "w_head")
        w_bdv = wtile(rr3(bdvp), [128, 4 * 64], "w_bdv")
        w_bd = wtile(rr3(bdp), [64, 7 * 64], "w_bd")
        w_bh128 = wtile(rr3(bh128p), [128, 7 * 128], "w_bh128")
        w_bh112 = wtile(rr3(bh112p), [112, 7 * 112], "w_bh112")
        w_bmv0 = wtile(bmv0p[:, :], [118, 112], "w_bmv0")
        w_bmv1 = wtile(bmv1p[:, :], [124, 112], "w_bmv1")
        w_bmv2 = wtile(bmv2p[:, :], [44, 32], "w_bmv2")
        w_cm = wtile(rr3(cmatp), [128, 49 * 98], "w_cm")
        w_bsel = wtile(bselp[:, :, :].rearrange("j u o -> u j o"), [8, 7 * 98], "w_bsel")
        w_asel = wtile(aselp[:, :], [2, 98], "w_asel")

        c_pre_b = wtile(pre_b[:, :], [128, 1], "c_pre_b", F32)
        c_fc1_g = wtile(fc1_g[:, :], [128, 1], "c_fc1_g", F32)
        c_fc1_be = wtile(fc1_be[:, :], [128, 1], "c_fc1_be", F32)
        c_fc2_b = wtile(fc2_b[:, :], [64, 1], "c_fc2_b", F32)
        c_e1_g = wtile(e1_g[:, :], [64, 1], "c_e1_g", F32)
        c_e1_be = wtile(e1_be[:, :], [64, 1], "c_e1_be", F32)
        c_e2_g = wtile(e2_g[:, :], [32, 1], "c_e2_g", F32)
        c_e2_be = wtile(e2_be[:, :], [32, 1], "c_e2_be", F32)
        c_e3_g = wtile(e3_g[:, :], [16, 1], "c_e3_g", F32)
        c_e3_be = wtile(e3_be[:, :], [16, 1], "c_e3_be", F32)
        c_head_b = wtile(head_b[:, :], [2, 1], "c_head_b", F32)
        c_m0 = wtile(m0p[:, :], [113, 1], "c_m0", F32)
        c_m1 = wtile(m1p[:, :], [113, 1], "c_m1", F32)

        c_bd2 = wtile(bd2p[:, :], [128, 2], "c_bd2")
        zt = wtile(zerop[:, :], [128, WP], "zt")

        zta = zt[:, :]
        def zrep(c, n, w):
            return bass.AP(zta.tensor, zta.offset,
                           [[zta.ap[0][0], c], [0, n], [1, w]])
        for buf, c in [(f1b, 64), (f2b, 64), (t1b, 128), (featb, 64), (f1sb, 64),
                       (lcb, 49), (bb, 1), (e1b, 64), (e2b, 32),
                       (e3b, 16), (headbuf, 2)]:
            nc.sync.dma_start(out=buf[:, 11:18, :], in_=zrep(c, 7, WP))
            nc.sync.dma_start(out=buf[:, :, 0:X0], in_=zrep(c, SLAB, X0))
            nc.sync.dma_start(out=buf[:, :, WP - 4:WP], in_=zrep(c, SLAB, 4))
        nc.sync.dma_start(out=f1b[:, 163:164, :], in_=zt[:64, :WP])

        st_fc1_s = statp.tile([128, 128], F32, name="st_fc1_s")
        st_fc1_q = statp.tile([128, 128], F32, name="st_fc1_q")
        st_e1_s = statp.tile([64, 128], F32, name="st_e1_s")
        st_e1_q = statp.tile([64, 128], F32, name="st_e1_q")
        st_e2_s = statp.tile([32, 128], F32, name="st_e2_s")
        st_e2_q = statp.tile([32, 128], F32, name="st_e2_q")
        st_e3_s = statp.tile([16, 128], F32, name="st_e3_s")
        st_e3_q = statp.tile([16, 128], F32, name="st_e3_q")
        st_c = statp.tile([98, 80], F32, name="st_c")
        for t in (st_fc1_s, st_fc1_q, st_e1_s, st_e1_q, st_e2_s, st_e2_q,
                  st_e3_s, st_e3_q, st_c):
            nc.vector.memset(t[:], 0.0)

        # ---------------- generic 3x3 conv pass (row-group batched DMA) ---
        def conv_pass(src_bufs, dst_buf, w_sb, cin, cout, dil, rows, tag,
                      bias=None, stats=None, norm=None, G=8, dsts=None):
            lo, hi = rows
            for g0 in range(lo, hi + 1, G):
                g1 = min(g0 + G - 1, hi)
                s0, s1 = g0 - dil, g1 + dil
                nr = s1 - s0 + 1
                t = rowp.tile([cin, nr * WP], F32R, name=f"{tag}_in",
                              tag="cin", bufs=2)
                p = 0
                for bsrc, c in src_bufs:
                    nc.sync.dma_start(out=t[p:p + c, :],
                                      in_=bsrc[:, s0:s1 + 1, :])
                    p += c
                if norm is not None:
                    for r in range(max(s0, 18), s1 + 1):
                        off = (r - s0) * WP
                        nc.scalar.activation(
                            t[:, off + X0:off + X0 + W],
                            t[:, off + X0:off + X0 + W], AF.Prelu,
                            bias=norm[1], scale=norm[0], alpha=0.2)
                ng = g1 - g0 + 1
                ot = outp.tile([cout, ng * W], F32R, name=f"{tag}_o",
                               tag="cout", bufs=2)
                for r in range(g0, g1 + 1):
                    ps = psp.tile([cout, W], F32, name=f"{tag}_ps", tag="ps")
                    for ti, (dy, dx) in enumerate(TAPS):
                        off = (r + dy * dil - s0) * WP
                        nc.tensor.matmul(
                            ps[:], w_sb[:, ti * cout:(ti + 1) * cout],
                            t[:, off + X0 + dx * dil:off + X0 + dx * dil + W],
                            start=(ti == 0), stop=(ti == 8))
                    o_sl = ot[:, (r - g0) * W:(r - g0 + 1) * W]
                    acc = None
                    if stats is not None and STAT_LO <= r <= STAT_HI:
                        acc = stats[0][:, r - STAT_LO:r - STAT_LO + 1]
                    if bias is not None:
                        nc.scalar.activation(o_sl, ps[:], AF.Identity,
                                             bias=bias, accum_out=acc)
                    else:
                        nc.scalar.activation(o_sl, ps[:], AF.Identity,
                                             accum_out=acc)
                    if stats is not None and STAT_LO <= r <= STAT_HI:
                        sq = outp.tile([cout, W], F32, name=f"{tag}_sq",
                                       tag="csq")
                        nc.scalar.activation(
                            sq[:], ps[:], AF.Square,
                            accum_out=stats[1][:, r - STAT_LO:r - STAT_LO + 1])
                if dsts is None:
                    nc.sync.dma_start(out=dst_buf[:, g0:g1 + 1, X0:X0 + W],
                                      in_=ot[:])
                else:
                    for db, pa, pb in dsts:
                        nc.sync.dma_start(out=db[:, g0:g1 + 1, X0:X0 + W],
                                          in_=ot[pa:pb, :])

        # ---------------- PASS 1: pre conv --------------------------------
        if upto >= 1:
            conv_pass([(f1in, 64), (f2in, 64)], None, w_pre, 128, 128, 1,
                      ROWS_F1, "pre", bias=c_pre_b[:, 0:1],
                      dsts=[(f1b, 0, 64), (f2b, 64, 128)])

        # ---------------- PASS 2: fc1 conv + stats ------------------------
        if upto >= 2:
            conv_pass([(f1b, 64), (f2b, 64)], t1b, w_fc1, 128, 128, 1, ROWS_T1, "fc1",
                      stats=(st_fc1_s, st_fc1_q))

        # ---------------- PASS 3: gaussian blur of f1, B ------------------
        if upto >= 3:
            lo, hi = ROWS_F1S
            GB = 8
            for c0 in range(lo, hi + 1, GB):
                c1 = min(c0 + GB - 1, hi)
                st0 = c0 - 3              # first pair-start loaded
                nst = c1 + 3 - st0 + 1    # pair-starts c0-3 .. c1+3
                gt = rowp.tile([128, nst * WP], F32R, name="f1p_in",
                               tag="pin", bufs=2)
                nc.sync.dma_start(
                    out=gt[:],
                    in_=bass.AP(f1b, st0 * WP,
                                [[WP, 2], [SLAB * WP, 64], [WP, nst], [1, WP]]))

                def fw(s):
                    o = (s - st0) * WP
                    return gt[:, o:o + WP]

                npair = (c1 - c0 + 1) // 2
                f1st_g = bigp.tile([128, npair * W], F32R, name="f1st",
                                   tag="f1stg", bufs=2)
                bt_g = outp.tile([2, npair * W], F32R, name="bt", tag="btg",
                                 bufs=2)
                vpair = None
                for r in range(c0, c1 + 1):
                    q = (r - lo) % 2
                    if q == 0:
                        vpair = bigp.tile([128, WP], F32R, name="vpair", tag="bigA")
                        nc.vector.tensor_copy(vpair[:, 0:X0], zt[:, 0:X0])
                        nc.vector.tensor_copy(vpair[:, WP - 4:WP], zt[:, 0:4])
                    psv = psp2.tile([64, W], F32, name="ps_v", tag="ps2")
                    if r < lo + 3:
                        for d in range(7):
                            ri = max(r + d - 3, lo)
                            nc.tensor.matmul(psv[:], w_bd[:, d * 64:(d + 1) * 64],
                                             fw(ri)[0:64, X0:X0 + W],
                                             start=(d == 0), stop=(d == 6))
                    else:
                        for t in range(4):
                            nc.tensor.matmul(psv[:], w_bdv[:, t * 64:(t + 1) * 64],
                                             fw(r - 3 + 2 * t)[:, X0:X0 + W],
                                             start=(t == 0), stop=(t == 3))
                    nc.scalar.activation(vpair[q * 64:(q + 1) * 64, X0:X0 + W],
                                         psv[:], AF.Identity)
                    if q == 1:
                        pi = (r - c0) // 2
                        nc.vector.tensor_copy(vpair[:, 1:4],
                                              vpair[:, X0:X0 + 1].to_broadcast([128, 3]))
                        nc.vector.tensor_copy(vpair[:, WP - 4:WP - 1],
                                              vpair[:, X0 + W - 1:X0 + W].to_broadcast([128, 3]))
                        psh = psp.tile([128, W], F32, name="ps_h", tag="ps")
                        for j in range(7):
                            nc.tensor.matmul(psh[:], w_bh128[:, j * 128:(j + 1) * 128],
                                             vpair[:, 1 + j:1 + j + W],
                                             start=(j == 0), stop=(j == 6))
                        nc.scalar.activation(f1st_g[:, pi * W:(pi + 1) * W],
                                             psh[:], AF.Identity)
                        sqt = bigp.tile([128, W], F32R, name="sqB", tag="bigB")
                        nc.scalar.activation(sqt[:], psh[:], AF.Square)
                        psb = psp2.tile([2, W], F32, name="ps_b", tag="ps2")
                        nc.tensor.matmul(psb[:], c_bd2[:], sqt[:], start=True,
                                         stop=True)
                        nc.scalar.activation(bt_g[:, pi * W:(pi + 1) * W],
                                             psb[:], AF.Identity)
                for q in range(2):
                    nc.sync.dma_start(
                        out=bass.AP(f1sb, (c0 + q) * WP + X0,
                                    [[SLAB * WP, 64], [2 * WP, npair], [1, W]]),
                        in_=f1st_g[q * 64:(q + 1) * 64, :])
                nc.sync.dma_start(
                    out=bass.AP(bb, c0 * WP + X0,
                                [[WP, 2], [2 * WP, npair], [1, W]]),
                    in_=bt_g[:])

        # ---------------- PASS 4: correlation (group-batched) -------------
        if upto >= 4:
            lo, hi = ROWS_CORR
            even_starts = list(range(lo, 155, 2))        # 69 pairs
            SPG = 4
            pidx = 0
            groups = [even_starts[i:i + SPG] for i in range(0, len(even_starts), SPG)]
            groups.append([155])
            for gl in groups:
                p0, pN = gl[0], gl[-1]
                st0 = p0 - 3
                nst = pN + 4 - st0 + 1
                stile = rowp.tile([128, nst * WP], F32R, name="f1sp_in",
                                  tag="pin", bufs=2)
                nc.sync.dma_start(
                    out=stile[:],
                    in_=bass.AP(f1sb, st0 * WP,
                                [[WP, 2], [SLAB * WP, 64], [WP, nst], [1, WP]]))
                npr = len(gl)
                f2g = rowp.tile([128, npr * WP], F32R, name="f2p", tag="aux",
                                bufs=2)
                for q in range(2):
                    nc.sync.dma_start(
                        out=f2g[q * 64:(q + 1) * 64, :],
                        in_=bass.AP(f2b, (p0 + q) * WP,
                                    [[SLAB * WP, 64], [2 * WP, npr], [1, WP]]))
                bwg = rowp.tile([8, npr * WP], F32R, name="bw8", tag="aux2",
                                bufs=2)
                nc.sync.dma_start(
                    out=bwg[:],
                    in_=bass.AP(bb, (p0 - 3) * WP,
                                [[WP, 8], [2 * WP, npr], [1, WP]]))
                ct_g = bigp.tile([98, npr * W], F32R, name="ct", tag="ctg",
                                 bufs=2)
                for gi, r in enumerate(gl):
                    f2p = f2g[:, gi * WP:(gi + 1) * WP]
                    sqa = bigp.tile([128, W], F32R, name="sqA", tag="bigB")
                    nc.scalar.activation(sqa[:], f2p[:, X0:X0 + W], AF.Square)
                    psa = psp2.tile([2, W], F32, name="ps_a", tag="ps2")
                    nc.tensor.matmul(psa[:], c_bd2[:], sqa[:], start=True, stop=True)
                    at = outp.tile([2, W], F32R, name="at", tag="small")
                    nc.scalar.activation(at[:], psa[:], AF.Identity)
                    psc = psp.tile([98, W], F32, name="ps_c", tag="ps")
                    # issue products (DVE: i<3, Pool: i>=3), then consume in an
                    # order interleaved by expected producer availability so PE
                    # rarely stalls on a single producer.
                    prods = {}
                    for i in range(7):
                        so = (r + i - 3 - st0) * WP
                        dve = i < 3
                        peng = nc.vector if dve else nc.gpsimd
                        for j in range(7):
                            kk = i * 7 + j
                            prod = bigp.tile([128, W], F32R, name="prod",
                                             tag="prodV" if dve else "prodP",
                                             bufs=4)
                            peng.tensor_tensor(
                                prod[:], f2p[:, X0:X0 + W],
                                stile[:, so + X0 + j - 3:so + X0 + j - 3 + W],
                                ALU.mult)
                            prods[kk] = prod
                    dve_ks = [i * 7 + j for i in range(3) for j in range(7)]
                    pool_ks = [i * 7 + j for i in range(3, 7) for j in range(7)]
                    order = []
                    di = pi2 = 0
                    while di < len(dve_ks) or pi2 < len(pool_ks):
                        tp = (di + 1) * 327 if di < len(dve_ks) else 1 << 30
                        tq = (pi2 + 1) * 213 if pi2 < len(pool_ks) else 1 << 30
                        if tq <= tp:
                            order.append(pool_ks[pi2]); pi2 += 1
                        else:
                            order.append(dve_ks[di]); di += 1
                    for oi, kk in enumerate(order):
                        nc.tensor.matmul(psc[:], w_cm[:, kk * 98:(kk + 1) * 98],
                                         prods[kk][:], start=(oi == 0), stop=False)
                    bo = gi * WP
                    for j in range(7):
                        nc.tensor.matmul(psc[:], w_bsel[:, j * 98:(j + 1) * 98],
                                         bwg[:, bo + X0 + j - 3:bo + X0 + j - 3 + W],
                                         start=False, stop=False)
                    nc.tensor.matmul(psc[:], w_asel[:], at[:], start=False, stop=True)
                    cred = outp.tile([98, 1], F32, name="cred", tag="small")
                    nc.vector.tensor_reduce(cred[:], psc[:], AX.X, ALU.add)
                    nc.vector.tensor_copy(st_c[:, pidx:pidx + 1], cred[:])
                    pidx += 1
                    nc.scalar.activation(ct_g[:, gi * W:(gi + 1) * W], psc[:],
                                         AF.Identity)
                cta = ct_g[:, :]
                for q in range(2):
                    nc.sync.dma_start(
                        out=bass.AP(lcb, (p0 + q) * WP + X0,
                                    [[SLAB * WP, 49], [2 * WP, npr], [1, W]]),
                        in_=bass.AP(cta.tensor, cta.offset + q * cta.ap[0][0],
                                    [[2 * cta.ap[0][0], 49], [W, npr], [1, W]]))

        # ---------------- AR1 ---------------------------------------------
        if upto >= 5:
            fs = statp.tile([128, 1], F32, name="fs")
            fq = statp.tile([128, 1], F32, name="fq")
            nc.vector.tensor_reduce(fs[:], st_fc1_s[:], AX.X, ALU.add)
            nc.vector.tensor_reduce(fq[:], st_fc1_q[:], AX.X, ALU.add)
            credp = statp.tile([98, 1], F32, name="credp")
            nc.vector.tensor_reduce(credp[:], st_c[:, 0:64], AX.X, ALU.add)
            # C partial: flatten [98,1] -> [1,98] via DRAM, reduce to scalar
            nc.sync.dma_start(out=scr[0:1, 0:98], in_=credp[:])
            ctot_r = statp.tile([1, 98], F32, name="ctot_r")
            nc.sync.dma_start(out=ctot_r[:], in_=scr[0:1, 0:98])
            ctot = statp.tile([1, 1], F32, name="ctot")
            nc.vector.tensor_reduce(ctot[:], ctot_r[:], AX.X, ALU.add)
            # payload: [0:128) sum, [128:256) sumsq, 256 = lc total
            nc.sync.dma_start(out=ar1i[0:1, 0:128], in_=fs[:])
            nc.sync.dma_start(out=ar1i[0:1, 128:256], in_=fq[:])
            nc.sync.dma_start(out=ar1i[0:1, 256:257], in_=ctot[:])
            if collectives:
                nc.gpsimd.collective_compute("AllReduce", ALU.add, replica_groups=GROUPS,
                                             ins=[ar1i[:, :]], outs=[ar1o[:, :]])
            else:
                nc.sync.dma_start(out=ar1o[:, :], in_=ar1i[:, :])
            fc1s = statp.tile([128, 1], F32, name="fc1s")
            fc1q = statp.tile([128, 1], F32, name="fc1q")
            nc.sync.dma_start(out=fc1s[:], in_=ar1o[0:1, 0:128])
            nc.sync.dma_start(out=fc1q[:], in_=ar1o[0:1, 128:256])
            sc = statp.tile([1, 1], F32, name="sc")
            nc.sync.dma_start(out=sc[:], in_=ar1o[0:1, 256:257])

            def norm_params(ssum, sq, gam, bet, n, c, nm):
                mu = statp.tile([c, 1], F32, name=f"mu_{nm}")
                nc.vector.tensor_scalar_mul(mu[:], ssum[:], 1.0 / n)
                var = statp.tile([c, 1], F32, name=f"var_{nm}")
                nc.vector.tensor_scalar_mul(var[:], sq[:], 1.0 / n)
                mu2 = statp.tile([c, 1], F32, name=f"mu2_{nm}")
                nc.vector.tensor_tensor(mu2[:], mu[:], mu[:], ALU.mult)
                nc.vector.tensor_tensor(var[:], var[:], mu2[:], ALU.subtract)
                nc.vector.tensor_scalar_add(var[:], var[:], EPS_IN)
                sd = statp.tile([c, 1], F32, name=f"sd_{nm}")
                nc.scalar.activation(sd[:], var[:], AF.Sqrt)
                rstd = statp.tile([c, 1], F32, name=f"rstd_{nm}")
                nc.vector.reciprocal(rstd[:], sd[:])
                s_ = statp.tile([c, 1], F32, name=f"s_{nm}")
                nc.vector.tensor_tensor(s_[:], gam[:], rstd[:], ALU.mult)
                b_ = statp.tile([c, 1], F32, name=f"b_{nm}")
                nc.vector.tensor_tensor(b_[:], mu[:], s_[:], ALU.mult)
                nc.vector.tensor_tensor(b_[:], bet[:], b_[:], ALU.subtract)
                return s_[:, 0:1], b_[:, 0:1]

            fc1_sn, fc1_bn = norm_params(fc1s, fc1q, c_fc1_g, c_fc1_be, 65536.0, 128, "fc1")

            # s = lcT / (49*H*W) + eps ; rs = 1/s ; scale e1 lc-channel weights
            sval = statp.tile([1, 1], F32, name="sval")
            nc.vector.tensor_scalar(sval[:], sc[0:1, 0:1], 1.0 / (49.0 * H * W), None,
                                    ALU.mult)
            nc.vector.tensor_scalar_add(sval[:], sval[:], EPS_CORR)
            rsv = statp.tile([1, 1], F32, name="rsv")
            nc.vector.reciprocal(rsv[:], sval[:])
            nc.sync.dma_start(out=scr[0:1, 100:101], in_=rsv[:])
            rb113 = statp.tile([113, 1], F32, name="rb113")
            nc.sync.dma_start(out=rb113[:],
                              in_=bass.AP(scr, 100, [[0, 113], [1, 1]]))
            fac = statp.tile([113, 1], F32, name="fac")
            nc.vector.tensor_tensor(fac[:], c_m1[:], rb113[:], ALU.mult)
            nc.vector.tensor_tensor(fac[:], fac[:], c_m0[:], ALU.add)
            nc.vector.tensor_scalar(w_e1[:], w_e1[:], fac[:, 0:1], None, ALU.mult)

        # ---------------- PASS 5: fc2 -------------------------------------
        if upto >= 6:
            conv_pass([(t1b, 128)], featb, w_fc2, 128, 64, 1, ROWS_FEAT, "fc2",
                      bias=c_fc2_b[:, 0:1], norm=(fc1_sn, fc1_bn))

        # ---------------- PASS 6: e1 --------------------------------------
        if upto >= 7:
            conv_pass([(featb, 64), (lcb, 49)], e1b, w_e1, 113, 64, 1, ROWS_E1,
                      "e1", stats=(st_e1_s, st_e1_q))

        # ---- AR2 / e2 / AR3 / e3 / AR4 / head ---------------------------
        if upto >= 8:
            def stats_ar(st_s, st_q, c, ari, aro, nm):
                s1 = statp.tile([c, 1], F32, name=f"ss_{nm}")
                q1 = statp.tile([c, 1], F32, name=f"qq_{nm}")
                nc.vector.tensor_reduce(s1[:], st_s[:], AX.X, ALU.add)
                nc.vector.tensor_reduce(q1[:], st_q[:], AX.X, ALU.add)
                nc.sync.dma_start(out=ari[0:1, 0:c], in_=s1[:])
                nc.sync.dma_start(out=ari[0:1, c:2 * c], in_=q1[:])
                if collectives:
                    nc.gpsimd.collective_compute("AllReduce", ALU.add,
                                                 replica_groups=GROUPS,
                                                 ins=[ari[:, :]], outs=[aro[:, :]])
                else:
                    nc.sync.dma_start(out=aro[:, :], in_=ari[:, :])
                s2 = statp.tile([c, 1], F32, name=f"ss2_{nm}")
                q2 = statp.tile([c, 1], F32, name=f"qq2_{nm}")
                nc.sync.dma_start(out=s2[:], in_=aro[0:1, 0:c])
                nc.sync.dma_start(out=q2[:], in_=aro[0:1, c:2 * c])
                return s2, q2

            e1s2, e1q2 = stats_ar(st_e1_s, st_e1_q, 64, ar2i, ar2o, "e1")
            e1_sn, e1_bn = norm_params(e1s2, e1q2, c_e1_g, c_e1_be, 65536.0, 64, "e1")

            conv_pass([(e1b, 64)], e2b, w_e2, 64, 32, 2, ROWS_E2, "e2",
                      stats=(st_e2_s, st_e2_q), norm=(e1_sn, e1_bn))
            e2s2, e2q2 = stats_ar(st_e2_s, st_e2_q, 32, ar3i, ar3o, "e2")
            e2_sn, e2_bn = norm_params(e2s2, e2q2, c_e2_g, c_e2_be, 65536.0, 32, "e2")

            conv_pass([(e2b, 32)], e3b, w_e3, 32, 16, 4, ROWS_E3, "e3",
                      stats=(st_e3_s, st_e3_q), norm=(e2_sn, e2_bn))
            e3s2, e3q2 = stats_ar(st_e3_s, st_e3_q, 16, ar4i, ar4o, "e3")
            e3_sn, e3_bn = norm_params(e3s2, e3q2, c_e3_g, c_e3_be, 65536.0, 16, "e3")

            conv_pass([(e3b, 16)], headbuf, w_head, 16, 2, 1, ROWS_HEAD, "head",
                      bias=c_head_b[:, 0:1], norm=(e3_sn, e3_bn))

        # ---------------- PASS 10: disp blur ------------------------------
        if upto >= 9:
            blocks = [
                (w_bmv0, 118, 18, 59, 18, 56, 112),
                (w_bmv1, 124, 71, 62, 74, 56, 112),
                (w_bmv2, 44, 127, 22, 130, 16, 32),
            ]
            for wm, kp, rin0, nin, ro0, nout, mp in blocks:
                ht = bigp.tile([kp, WP], F32R, name="dh_in", tag="bigA")
                nc.sync.dma_start(
                    out=ht[:],
                    in_=headbuf[:, rin0:rin0 + nin, :].rearrange("c r x -> r c x"))
                psv = psp.tile([mp, WP], F32, name="dps_v", tag="ps")
                nc.tensor.matmul(psv[:], wm[:], ht[:], start=True, stop=True)
                vt = bigp.tile([mp, WP], F32R, name="dvt", tag="bigB")
                nc.scalar.activation(vt[:], psv[:], AF.Identity)
                nc.vector.tensor_copy(vt[:, 1:4], vt[:, X0:X0 + 1].to_broadcast([mp, 3]))
                nc.vector.tensor_copy(vt[:, WP - 4:WP - 1],
                                      vt[:, X0 + W - 1:X0 + W].to_broadcast([mp, 3]))
                psh = psp.tile([mp, W], F32, name="dps_h", tag="ps")
                for j in range(7):
                    nc.tensor.matmul(psh[:], w_bh112[0:mp, j * 112:j * 112 + mp],
                                     vt[:, 1 + j:1 + j + W], start=(j == 0), stop=(j == 6))
                ot = outp.tile([mp, W], F32, name="do", tag="small")
                nc.scalar.activation(ot[:], psh[:], AF.Identity)
                nc.sync.dma_start(
                    out=out[:, ro0 - R0:ro0 - R0 + nout, :].rearrange("c r x -> r c x"),
                    in_=ot[:])

    _fix_waits(nc)
    return nc


# ---------------------------------------------------------------- entry
_NC = None


def kernel(**inputs):
    global _NC
    if _NC is None:
        _NC = build_module()
    in_maps = [_prep_core_inputs(inputs, c) for c in range(8)]
    res = run_bass_kernel_spmd(_NC, in_maps, list(range(8)))
    disp = np.zeros((B, 2, H, W), np.float32)
    for c in range(8):
        b, h = c // 2, c % 2
        o = res.results[c]["out"]
        if h == 0:
            disp[b, :, 0:HALF, :] = o
        else:
            disp[b, :, HALF:H, :] = o[:, ::-1, :]
    return disp

